# revision 1
# baseline (speedup 1.0000x reference)
"""KANLinear fused kernel for 8x Trainium2 NeuronCores.

out[b,o] = silu(x) @ Wb^T + einsum('bik,oik->bo', bspline_basis(x), Ws)

Data-parallel over the 8192-token batch (1024 rows/core).

Algebraic re-basis of the spline (exact): with zp = clip(x,-1,1)+1 in [0,2],
z = zp-1, the 8 cubic B-spline basis functions lie in the span of
  {1, z, z^2, z^3, E1, E2, E3, E4},  Ek = relu(+-2.5*zp + c_k)^3
(truncated-power basis of the C2 piecewise-cubic space; the two basis
functions at each boundary ARE single truncated powers, the central four
are reproduced by the global cubic). The 8x8 change-of-basis M is folded
into the spline weights on the host; the constant-feature column becomes a
per-output bias. Per-core contraction: K = 8*1024 (64 chunks of 128):
  per input-feature chunk f: rows = [silu(x); z; z^2; z^3; E1..E4]
Feature compute is ~19 cheap ACT/DVE ops per chunk (no masks/one-hots).

Two passes over batch halves (512 rows each). PSUM = 8 banks =
4 batch-tiles x 2 output-halves accumulating over all 64 K-chunks.
Features are the matmul stationary operand (one LDWEIGHTS feeds 2
matmuls); weights stream from DRAM (256 KB per K-chunk). Dummy warmup
matmuls keep the PE clock-gate at full rate through the preamble; x and
the output are staged fp16 (the host upcasts to f32 and adds the folded
bias). The final pass processes its last K-chunk bank-major so PSUM
banks complete staggered and 7 of 8 output drains overlap the matmuls.
"""
import sys
if "/opt/trn_rl_repo" not in sys.path:
    sys.path.insert(0, "/opt/trn_rl_repo")

import numpy as np
import concourse.bass as bass
from concourse import bacc
import concourse.tile as tile
import concourse.mybir as mybir
from concourse.bass_utils import run_bass_kernel_spmd

AF = mybir.ActivationFunctionType
OP = mybir.AluOpType
F32, F16 = mybir.dt.float32, mybir.dt.float16

N_CORES = 8
B_FULL, I_FEAT, O_FEAT = 8192, 1024, 1024
B_LOC = B_FULL // N_CORES          # 1024 batch rows per core
BH = B_LOC // 2                    # 512 rows per pass
N_CHUNK = I_FEAT // 128            # 8 input-feature chunks
N_KC = 8 * N_CHUNK                 # 64 contraction chunks of 128

_COMPILED = None


def _build_program():
    nc = bacc.Bacc("TRN2", target_bir_lowering=False, debug=False)
    xT = nc.dram_tensor("xT", [I_FEAT, B_LOC], F16, kind="ExternalInput").ap()
    wt = nc.dram_tensor("wt", [N_KC, 128, O_FEAT], F16, kind="ExternalInput").ap()
    out = nc.dram_tensor("out", [B_LOC, O_FEAT], F16, kind="ExternalOutput").ap()

    dve, act = nc.vector, nc.scalar

    # activation() resolves float bias/scale via the const-AP registry;
    # register the ones this kernel uses (mirrors Bass.__init__).
    def reg_const(v):
        key = (F32, float(v))
        if key not in nc.const_aps.aps:
            t = nc.alloc_sbuf_tensor(f"constk-{len(nc.const_aps.aps)}", [128, 1], F32)
            nc.gpsimd.memset(t.ap(), float(v))
            nc.const_aps.aps[key] = t.ap()
    for v in (-1.0, 2.0, -2.5, 2.5, -3.0, -4.0):
        reg_const(v)
    # No barrier: the first activation that reads these consts (r1) sits on
    # the ACT queue behind silu, which waits on the first x DMA (~4us after
    # the gpsimd memsets above complete at the head of the GPS queue).

    with tile.TileContext(nc) as tc:
        with tc.tile_pool(name="xin", bufs=2) as xpool, \
             tc.tile_pool(name="mid", bufs=2) as mid, \
             tc.tile_pool(name="feat", bufs=2) as fpool, \
             tc.tile_pool(name="wstream", bufs=10) as wstream, \
             tc.tile_pool(name="warm", bufs=1) as wpool, \
             tc.tile_pool(name="outsb", bufs=4) as opool, \
             tc.tile_pool(name="psum", bufs=1, space="PSUM") as pspool:

            # HAM warmup: ~6us of dummy matmuls so the PE clock-gate is at
            # 8/8 by the time the first real matmul's operands arrive.
            warm16 = wpool.tile([128, 512], F16, tag="wrm", name="wrm")
            nc.gpsimd.memset(warm16[:], 0.0)
            warm_ps = pspool.tile([128, 512], F32, tag="ps0", name="ps0w")
            for _ in range(14):
                nc.tensor.matmul(warm_ps[:], warm16[:, 0:128], warm16[:],
                                 start=True, stop=True)

            for bh in range(2):
                psums = [pspool.tile([128, 512], F32, tag=f"ps{j}", name=f"ps{j}")
                         for j in range(8)]   # j = bt*2 + oh
                tail_wts = [None] * 8   # final-pass: last chunk's weight tiles
                tail_feats = None
                for f in range(N_CHUNK):
                    xin = xpool.tile([128, BH], F16, tag="x", name="x")
                    nc.sync.dma_start(
                        xin[:], xT[f * 128:(f + 1) * 128, bh * BH:(bh + 1) * BH])

                    feats = [None] * 8
                    # r=0: silu feature row
                    sl = fpool.tile([128, BH], F16, tag="silu", name="silu")
                    act.activation(sl[:], xin[:], AF.Silu)
                    feats[0] = sl

                    # zp = clip(x,-1,1)+1 = relu(2 - relu(1-x))   (kept f32)
                    r1 = mid.tile([128, BH], F32, tag="r1", name="r1")
                    act.activation(r1[:], xin[:], AF.Relu, scale=-1.0, bias=1.0)
                    zp = mid.tile([128, BH], F32, tag="zp", name="zp")
                    act.activation(zp[:], r1[:], AF.Relu, scale=-1.0, bias=2.0)

                    # r=1..3: z, z^2, z^3
                    z = fpool.tile([128, BH], F16, tag="z", name="z")
                    act.activation(z[:], zp[:], AF.Copy, scale=1.0, bias=-1.0)
                    feats[1] = z
                    z2 = fpool.tile([128, BH], F16, tag="z2", name="z2")
                    dve.tensor_tensor(z2[:], z[:], z[:], OP.mult)
                    feats[2] = z2
                    z3 = fpool.tile([128, BH], F16, tag="z3", name="z3")
                    dve.tensor_tensor(z3[:], z2[:], z[:], OP.mult)
                    feats[3] = z3

                    # r=4..7: Ek = relu(+-2.5*zp + c)^3
                    for r, (sc, cb) in enumerate(
                            ((-2.5, 1.0), (-2.5, 2.0), (2.5, -3.0), (2.5, -4.0)),
                            start=4):
                        a = mid.tile([128, BH], F16, tag=f"a{r}", name=f"a{r}")
                        act.activation(a[:], zp[:], AF.Relu, scale=sc, bias=cb)
                        q = mid.tile([128, BH], F16, tag=f"q{r}", name=f"q{r}")
                        act.activation(q[:], a[:], AF.Square)
                        c = fpool.tile([128, BH], F16, tag=f"c{r}", name=f"c{r}")
                        dve.tensor_tensor(c[:], q[:], a[:], OP.mult)
                        feats[r] = c

                    # matmuls for this chunk: 8 kc, features stationary.
                    # In the final pass the last chunk's matmuls are deferred
                    # to a bank-major tail (below) so PSUM banks complete
                    # staggered and their drains overlap the matmul stream.
                    for r in range(8):
                        kc = 8 * f + r
                        wts = wstream.tile([128, O_FEAT], F16, tag="w", name="w")
                        nc.sync.dma_start(wts[:], wt[kc, :, :])
                        if f == N_CHUNK - 1:
                            tail_wts[r] = wts
                            continue
                        fsb = feats[r]
                        for bt in range(4):
                            lhsT = fsb[:, bt * 128:(bt + 1) * 128]
                            for oh in range(2):
                                nc.tensor.matmul(
                                    psums[bt * 2 + oh][:],
                                    lhsT,
                                    wts[:, oh * 512:(oh + 1) * 512],
                                    start=(kc == 0), stop=False,
                                )
                    if f == N_CHUNK - 1:
                        tail_feats = feats

                # bank-major tail over the last chunk (both passes): bank j
                # receives its final 8 contributions, then drains immediately
                # while the PE works on banks j+1..7. Pass-1 drains overlap
                # pass 2; pass-2 drains overlap the end of the stream.
                for j in range(8):
                    bt, oh = j // 2, j % 2
                    for r in range(8):
                        nc.tensor.matmul(
                            psums[j][:],
                            tail_feats[r][:, bt * 128:(bt + 1) * 128],
                            tail_wts[r][:, oh * 512:(oh + 1) * 512],
                            start=False, stop=(r == 7),
                        )
                    rows = slice(bh * BH + bt * 128, bh * BH + (bt + 1) * 128)
                    cols = slice(oh * 512, (oh + 1) * 512)
                    ob = opool.tile([128, 512], F16, tag=f"obt{oh}",
                                    name=f"obt{oh}")
                    if oh == 0:
                        dve.tensor_copy(ob[:], psums[j][:])
                    else:
                        act.activation(ob[:], psums[j][:], AF.Copy)
                    nc.sync.dma_start(out[rows, cols], ob[:])
    nc.compile()
    return nc


def _get_program():
    global _COMPILED
    if _COMPILED is None:
        _COMPILED = _build_program()
    return _COMPILED


def _basis_fold_matrix():
    """M (8 features x 8 basis) s.t. basis_g(x) = sum_f M[f,g] * F_f(x)."""
    G = np.linspace(-2.2, 2.2, 12)
    xs = np.linspace(-4.0, 4.0, 40001)
    xc = np.clip(xs, -1.0, 1.0)
    xg = xc[..., None]
    basis = ((xg >= G[:-1]) & (xg < G[1:])).astype(np.float64)
    for k in range(1, 4):
        ld = G[k:-1] - G[:-(k + 1)]; ld = np.where(ld == 0, 1, ld)
        rd = G[k + 1:] - G[1:-k]; rd = np.where(rd == 0, 1, rd)
        basis = ((xg - G[:-(k + 1)]) / ld * basis[..., :-1]
                 + (G[k + 1:] - xg) / rd * basis[..., 1:])
    zp = xc + 1.0
    z = zp - 1.0
    F = np.stack([np.ones_like(z), z, z * z, z ** 3,
                  np.maximum(1 - 2.5 * zp, 0) ** 3,
                  np.maximum(2 - 2.5 * zp, 0) ** 3,
                  np.maximum(2.5 * zp - 3, 0) ** 3,
                  np.maximum(2.5 * zp - 4, 0) ** 3], axis=-1)
    M, *_ = np.linalg.lstsq(F, basis, rcond=None)
    return M  # (8, 8)


def _prep_weights(base_weight, spline_weight):
    bw = np.ascontiguousarray(base_weight, dtype=np.float64)
    sw = np.ascontiguousarray(spline_weight, dtype=np.float64)
    M = _basis_fold_matrix()
    Wt = np.einsum('oig,fg->oif', sw, M)            # (O, I, 8)
    bias = Wt[:, :, 0].sum(axis=1)                  # (O,)
    # wt[kc = 8f+r, kr, o]: r=0 silu -> bw rows; r>=1 -> Wt[..., r]
    wt = np.empty((N_KC, 128, O_FEAT), dtype=np.float32)
    for f in range(N_CHUNK):
        rows = slice(f * 128, (f + 1) * 128)
        wt[8 * f + 0] = bw.T[rows, :]
        for r in range(1, 8):
            wt[8 * f + r] = Wt[:, rows, r].T
    return wt.astype(np.float16), bias.astype(np.float32)


def _run(x, base_weight, spline_weight, trace=False, tmpdir=None):
    nc = _get_program()
    x = np.ascontiguousarray(x, dtype=np.float16)
    wt16, bias = _prep_weights(base_weight, spline_weight)
    in_maps = []
    for c in range(N_CORES):
        xc = np.ascontiguousarray(x[c * B_LOC:(c + 1) * B_LOC, :].T)
        in_maps.append({"xT": xc, "wt": wt16})
    res = run_bass_kernel_spmd(nc, in_maps, core_ids=list(range(N_CORES)),
                               trace=trace, tmpdir=tmpdir)
    full = np.concatenate([res.results[c]["out"] for c in range(N_CORES)],
                          axis=0).astype(np.float32)
    full += bias[None, :]
    return full, res


def kernel(x, base_weight, spline_weight):
    out, _ = _run(x, base_weight, spline_weight, trace=False)
    return out

